# revision 27
# baseline (speedup 1.0000x reference)
"""Trainium2 Bass kernel for BinaryRelativePositionEmbedding.

Math: out[b,h,l,m] = q[b,h,l,:] . rp[m,:],  rp = bits @ emb, where
bits[m,:] are the 12 two's-complement bits of position (m - L + 1).

Key identity: out[l, m] = sum_b bits[m,b] * s[l,b] with s = q @ emb^T
(rank 12).  The pattern v(m) = (m - (L-1)) & 4095 ranges over all 12-bit
values except 2048, so each row-tile of the output is a subset-sum table
over the 12 per-row scalars s[l, :], built with doubling steps on the
vector engine.  The table is laid out rotated by 2048 so the final
output row is the single contiguous slice U[:, 1:4096]:
    U[:, 2048+w] = subset-sum of bits 0..10 over w   (w in [0,2048))
    U[:, c]      = U[:, 2048+c] + s_11               (c in [0,2048))
    => U[:, 1+m] = T[(m + 2049) & 4095] = out[:, m]  (m in [0,4095))
Tables for the two rows a partition holds are PACKED at stride 4095
(element 0 of a table is never read or written), so each partition is
one contiguous 32760-byte DMA descriptor covering both of its rows.

All output DMAs go on ONE HWDGE ring (nc.sync): when two rings hold
backlog concurrently, each SDMA engine round-robins between them at
packet granularity and per-packet time degrades 629ns -> 824ns (+31%).
A single FIFO ring sustains ~425 GB/s (617ns per 16380B packet, 97.6%
of the 435 GB/s SBUF-AXI fabric ceiling) for the whole drain.

Startup: PSUM groups are split [2,2,4,8,...] with the tiny s-copies
on the SCALAR engine, so the first table build depends only on two
matmuls and the DVE critical path is builds-only (the Tile scheduler
otherwise hoists copies + their matmul waits between early builds,
starving the ring).  Keep the structure EXACTLY as is: psum bufs=2,
tab bufs=3, 2-tile batches, no scalar-engine builds, no extra priming
DMAs.  Ten experiments show this is a narrow local optimum -- any
perturbation (psum bufs=8, tab bufs=4/6, 1-tile batches, scalar-engine
table builds, half-batch priming DMAs) locks the SDMA engines into a
degraded regime (860-1680ns packets or periodic ring starvation),
costing +20-38%% exec.

Sharding: data-parallel over the 32 (b,h) pairs, 4 per NeuronCore.
"""

import os
import sys

import numpy as np

if "/opt/trn_rl_repo" not in sys.path:
    sys.path.insert(0, "/opt/trn_rl_repo")

import concourse.bass as bass  # noqa: E402
import concourse.mybir as mybir  # noqa: E402
from concourse import bacc, tile  # noqa: E402
from concourse.bass_utils import run_bass_kernel_spmd  # noqa: E402

F32 = mybir.dt.float32

B, H, L, D = 2, 16, 2048, 64
NB = 12                  # bits per position
M = 2 * L - 1            # 4095 relative positions
NCORES = 8
PAIRS = B * H            # 32
PPC = PAIRS // NCORES    # 4 (b,h) pairs per core
ROWS = PPC * L           # 8192 output rows per core
NT = ROWS // 128         # 64 row-tiles

# PSUM sub-groups: first tiles get their own psum tile + copy so the
# first build only waits on two matmuls.  Builds go in 2-tile batches:
# per-batch fixed overhead (semaphore + sequencer dead time) is ~1.2us
# regardless of batch size, so 1-tile batches make the DVE the pacer
# (+56us vector time, exec 411us vs 358us).
GROUPS = [2, 2, 4] + [8] * 7


LAST_EXEC_TIME_NS = None


def _build_nc():
    nc = bacc.Bacc(None)
    qT = nc.declare_dram_parameter("qT", [D, ROWS], F32, isOutput=False)
    embT = nc.declare_dram_parameter("embT", [D, NB], F32, isOutput=False)
    out = nc.declare_dram_parameter("out", [ROWS, M], F32, isOutput=True)

    # input chunks: first two tiles alone so the first matmuls start
    # ASAP, then the next 6 tiles, then 8-tile chunks.
    chunk_tiles = [2, 6] + [8] * 7
    chunks = []
    c0 = 0
    for n in chunk_tiles:
        chunks.append((c0, n * 128))
        c0 += n * 128

    def chunk_of(tile_idx):
        c0 = 0
        for ci, n in enumerate(chunk_tiles):
            if tile_idx < c0 + n:
                return ci, c0
            c0 += n
        raise AssertionError

    with tile.TileContext(nc) as tc:
        with (
            tc.tile_pool(name="const", bufs=1) as cpool,
            tc.tile_pool(name="psum", bufs=2, space="PSUM") as ppool,
            tc.tile_pool(name="tab", bufs=3) as tpool,
        ):
            embt_sb = cpool.tile([D, NB], F32)
            s_sb = cpool.tile([128, NT * NB], F32)
            minis = [
                cpool.tile([128, 4096], F32, name=f"mini{j}", tag=f"mini{j}")
                for j in range(2)
            ]
            qt_chunks = [
                cpool.tile([D, csz], F32, name=f"qt{g}", tag=f"qt{g}")
                for g, (_, csz) in enumerate(chunks)
            ]

            nc.scalar.dma_start(out=embt_sb[:], in_=embT[:])
            for g, (c0, csz) in enumerate(chunks):
                nc.scalar.dma_start(out=qt_chunks[g][:], in_=qT[:, c0 : c0 + csz])

            t0 = 0
            for grp_n in GROUPS:
                grp = list(range(t0, t0 + grp_n))
                ps = ppool.tile([128, grp_n * NB], F32, name="ps", tag="ps")
                for j, t in enumerate(grp):
                    ci, ct0 = chunk_of(t)
                    off = (t - ct0) * 128
                    nc.tensor.matmul(
                        ps[:, j * NB : (j + 1) * NB],
                        lhsT=qt_chunks[ci][:, off : off + 128],
                        rhs=embt_sb[:],
                        start=True,
                        stop=True,
                    )
                # s[l, b] = q[l, :] . emb[b, :]; copy on the scalar engine
                # keeps the DVE stream builds-only.
                nc.scalar.copy(
                    out=s_sb[:, t0 * NB : (t0 + grp_n) * NB],
                    in_=ps[:, : grp_n * NB],
                )

                if t0 == 0:
                    # Prime the ring with two HALF-batches: build tile 0,
                    # DMA its 128 rows immediately (even DRAM rows of
                    # super-tile 0 -> stride-2 dst), then tile 1 (odd rows).
                    # First descriptors reach the ring ~5us sooner than
                    # waiting for a full 2-tile build.  Unpacked mini
                    # tables in persistent buffers (no pool reuse stall).
                    dst3 = out[0:256, :].rearrange("(p r) m -> p r m", p=128)
                    for j in range(2):
                        mini = minis[j]
                        sb = j * NB
                        nc.vector.memset(mini[:, 2048:2049], 0.0)
                        nc.vector.tensor_copy(
                            out=mini[:, 2049:2050], in_=s_sb[:, sb : sb + 1]
                        )
                        for k in range(1, NB - 1):
                            nc.vector.tensor_scalar_add(
                                mini[:, 2048 + 2**k : 2048 + 2 ** (k + 1)],
                                mini[:, 2048 : 2048 + 2**k],
                                s_sb[:, sb + k : sb + k + 1],
                            )
                        nc.vector.tensor_scalar_add(
                            mini[:, 1:2048],
                            mini[:, 2049:4096],
                            s_sb[:, sb + NB - 1 : sb + NB],
                        )
                        src = mini[:, 1:4096].rearrange("p (x m) -> p x m", x=1)
                        nc.sync.dma_start(out=dst3[:, j : j + 1, :], in_=src)
                    batch_starts = range(t0 + 2, t0 + grp_n, 2)
                else:
                    batch_starts = range(t0, t0 + grp_n, 2)
                for b0 in batch_starts:
                    # One buffer covers s-tiles (b0, b0+1) = super-tile b0//2
                    # in the PACKED layout: the two tables sit at stride 4095
                    # (element 0 of each table is never read or written, so
                    # consecutive tables overlap by one column).  Partition p
                    # then holds rows p*2 and p*2+1 of the super-tile as ONE
                    # contiguous 32760B span -> one DMA descriptor per
                    # partition instead of two, amortizing the ~15ns/packet
                    # SDMA overhead (617 -> ~610 ns per 16380B equivalent).
                    U = tpool.tile([128, 2 * 4095 + 1], F32, name="U", tag="U")
                    for j, ti in enumerate([b0, b0 + 1]):
                        sb = ti * NB
                        base = j * 4095
                        hi = base + 2048
                        nc.vector.memset(U[:, hi : hi + 1], 0.0)
                        nc.vector.tensor_copy(
                            out=U[:, hi + 1 : hi + 2], in_=s_sb[:, sb : sb + 1]
                        )
                        for k in range(1, NB - 1):
                            nc.vector.tensor_scalar_add(
                                U[:, hi + 2**k : hi + 2 ** (k + 1)],
                                U[:, hi : hi + 2**k],
                                s_sb[:, sb + k : sb + k + 1],
                            )
                        nc.vector.tensor_scalar_add(
                            U[:, base + 1 : base + 2048],
                            U[:, hi + 1 : hi + 2048],
                            s_sb[:, sb + NB - 1 : sb + NB],
                        )
                    r0 = b0 * 128
                    src = U[:, 1 : 2 * 4095 + 1]
                    dst = out[r0 : r0 + 256, :].rearrange("(p r) m -> p (r m)", p=128)
                    nc.sync.dma_start(out=dst, in_=src)
                t0 += grp_n

    nc.finalize()
    return nc


def _install_trace_shim():
    """Make run_bass_kernel_spmd(trace=True) work under axon in this
    container: provide antenv.axon_hooks backed by ctypes calls into
    libaxon_pjrt.so, and skip the S3 artifact upload."""
    import contextlib
    import ctypes
    import types

    import antenv
    from concourse import bass_utils

    if getattr(antenv, "axon_hooks", None) is not None:
        return

    def _ntff_profile_via_ctypes(so_path):
        lib = ctypes.CDLL(so_path)
        if not hasattr(lib, "axon_start_nrt_profile"):
            return None
        lib.axon_start_nrt_profile.argtypes = [
            ctypes.POINTER(ctypes.c_int64),
            ctypes.c_size_t,
        ]
        lib.axon_start_nrt_profile.restype = ctypes.c_int64
        lib.axon_stop_nrt_profile.argtypes = [ctypes.c_char_p]
        lib.axon_stop_nrt_profile.restype = ctypes.c_int64

        @contextlib.contextmanager
        def _hook(output_dir, device_ids):
            import jax

            jax.devices()
            if device_ids:
                ids = (ctypes.c_int64 * len(device_ids))(*device_ids)
                rc = lib.axon_start_nrt_profile(ids, len(device_ids))
            else:
                rc = lib.axon_start_nrt_profile(None, 0)
            if rc != 0:
                raise RuntimeError(f"axon_start_nrt_profile rc={rc}")
            try:
                yield
            finally:
                n = lib.axon_stop_nrt_profile(str(output_dir).encode())
                print(f"trace shim: {n} ntff file(s) in {output_dir}", file=sys.stderr)

        return _hook

    mod = types.ModuleType("antenv.axon_hooks")
    state = {"hook": _ntff_profile_via_ctypes("/opt/axon/libaxon_pjrt.so")}
    mod.set_axon_ntff_profile_hook = lambda h: state.__setitem__("hook", h)
    mod.get_axon_ntff_profile_hook = lambda: state["hook"]
    sys.modules["antenv.axon_hooks"] = mod
    antenv.axon_hooks = mod
    bass_utils.upload_artifacts = lambda tmpdir: f"local://{tmpdir}"


def kernel(q, k, emb):
    global LAST_EXEC_TIME_NS
    trace = os.environ.get("KERNEL_TRACE", "") == "1"
    if trace:
        _install_trace_shim()

    nc = _build_nc()

    qr = np.asarray(q, dtype=np.float32).reshape(PAIRS, L, D)
    embT = np.ascontiguousarray(np.asarray(emb, dtype=np.float32).T)
    # Packed-layout column permutation: matmul for s-tile tt must place,
    # at PSUM partition p, the scalars of output row st*256 + p*2 + r
    # (st = tt//2, r = tt%2), so qT column tt*128+p holds that q row.
    perm = np.empty(ROWS, dtype=np.int64)
    p_ar = np.arange(128)
    for tt in range(NT):
        st, r = divmod(tt, 2)
        perm[tt * 128 + p_ar] = st * 256 + p_ar * 2 + r
    in_maps = []
    for c in range(NCORES):
        qc = qr[c * PPC : (c + 1) * PPC]  # [PPC, L, D]
        qTc = qc.transpose(2, 0, 1).reshape(D, ROWS)
        qTc = np.ascontiguousarray(qTc[:, perm])
        in_maps.append({"qT": qTc, "embT": embT})

    res = run_bass_kernel_spmd(nc, in_maps, core_ids=list(range(NCORES)), trace=trace)
    LAST_EXEC_TIME_NS = res.exec_time_ns

    out = np.empty((PAIRS, L, M), np.float32)
    for c in range(NCORES):
        out[c * PPC : (c + 1) * PPC] = res.results[c]["out"].reshape(PPC, L, M)
    return out.reshape(B, H, L, M)


# revision 28
# speedup vs baseline: 1.0012x; 1.0012x over previous
"""Trainium2 Bass kernel for BinaryRelativePositionEmbedding.

Math: out[b,h,l,m] = q[b,h,l,:] . rp[m,:],  rp = bits @ emb, where
bits[m,:] are the 12 two's-complement bits of position (m - L + 1).

Key identity: out[l, m] = sum_b bits[m,b] * s[l,b] with s = q @ emb^T
(rank 12).  The pattern v(m) = (m - (L-1)) & 4095 ranges over all 12-bit
values except 2048, so each row-tile of the output is a subset-sum table
over the 12 per-row scalars s[l, :], built with doubling steps on the
vector engine.  The table is laid out rotated by 2048 so the final
output row is the single contiguous slice U[:, 1:4096]:
    U[:, 2048+w] = subset-sum of bits 0..10 over w   (w in [0,2048))
    U[:, c]      = U[:, 2048+c] + s_11               (c in [0,2048))
    => U[:, 1+m] = T[(m + 2049) & 4095] = out[:, m]  (m in [0,4095))
Tables for the two rows a partition holds are PACKED at stride 4095
(element 0 of a table is never read or written), so each partition is
one contiguous 32760-byte DMA descriptor covering both of its rows.

All output DMAs go on ONE HWDGE ring (nc.sync): when two rings hold
backlog concurrently, each SDMA engine round-robins between them at
packet granularity and per-packet time degrades 629ns -> 824ns (+31%).
A single FIFO ring sustains ~425 GB/s (617ns per 16380B packet, 97.6%
of the 435 GB/s SBUF-AXI fabric ceiling) for the whole drain.

Startup: PSUM groups are split [2,2,4,8,...] with the tiny s-copies
on the SCALAR engine, so the first table build depends only on two
matmuls and the DVE critical path is builds-only (the Tile scheduler
otherwise hoists copies + their matmul waits between early builds,
starving the ring).  Keep the structure EXACTLY as is: psum bufs=2,
tab bufs=3, 2-tile batches, no scalar-engine builds, no extra priming
DMAs.  Ten experiments show this is a narrow local optimum -- any
perturbation (psum bufs=8, tab bufs=4/6, 1-tile batches, scalar-engine
table builds, half-batch priming DMAs) locks the SDMA engines into a
degraded regime (860-1680ns packets or periodic ring starvation),
costing +20-38%% exec.

Sharding: data-parallel over the 32 (b,h) pairs, 4 per NeuronCore.
"""

import os
import sys

import numpy as np

if "/opt/trn_rl_repo" not in sys.path:
    sys.path.insert(0, "/opt/trn_rl_repo")

import concourse.bass as bass  # noqa: E402
import concourse.mybir as mybir  # noqa: E402
from concourse import bacc, tile  # noqa: E402
from concourse.bass_utils import run_bass_kernel_spmd  # noqa: E402

F32 = mybir.dt.float32

B, H, L, D = 2, 16, 2048, 64
NB = 12                  # bits per position
M = 2 * L - 1            # 4095 relative positions
NCORES = 8
PAIRS = B * H            # 32
PPC = PAIRS // NCORES    # 4 (b,h) pairs per core
ROWS = PPC * L           # 8192 output rows per core
NT = ROWS // 128         # 64 row-tiles

# PSUM sub-groups: first tiles get their own psum tile + copy so the
# first build only waits on two matmuls.  Builds go in 2-tile batches:
# per-batch fixed overhead (semaphore + sequencer dead time) is ~1.2us
# regardless of batch size, so 1-tile batches make the DVE the pacer
# (+56us vector time, exec 411us vs 358us).
GROUPS = [2, 2, 4] + [8] * 7


LAST_EXEC_TIME_NS = None


def _build_nc():
    nc = bacc.Bacc(None)
    qT = nc.declare_dram_parameter("qT", [D, ROWS], F32, isOutput=False)
    embT = nc.declare_dram_parameter("embT", [D, NB], F32, isOutput=False)
    out = nc.declare_dram_parameter("out", [ROWS, M], F32, isOutput=True)

    # input chunks: first two tiles alone so the first matmuls start
    # ASAP, then the next 6 tiles, then 8-tile chunks.
    chunk_tiles = [2, 6] + [8] * 7
    chunks = []
    c0 = 0
    for n in chunk_tiles:
        chunks.append((c0, n * 128))
        c0 += n * 128

    def chunk_of(tile_idx):
        c0 = 0
        for ci, n in enumerate(chunk_tiles):
            if tile_idx < c0 + n:
                return ci, c0
            c0 += n
        raise AssertionError

    with tile.TileContext(nc) as tc:
        with (
            tc.tile_pool(name="const", bufs=1) as cpool,
            tc.tile_pool(name="psum", bufs=2, space="PSUM") as ppool,
            tc.tile_pool(name="tab", bufs=3) as tpool,
        ):
            embt_sb = cpool.tile([D, NB], F32)
            s_sb = cpool.tile([128, NT * NB], F32)
            qt_chunks = [
                cpool.tile([D, csz], F32, name=f"qt{g}", tag=f"qt{g}")
                for g, (_, csz) in enumerate(chunks)
            ]

            nc.scalar.dma_start(out=embt_sb[:], in_=embT[:])
            for g, (c0, csz) in enumerate(chunks):
                nc.scalar.dma_start(out=qt_chunks[g][:], in_=qT[:, c0 : c0 + csz])

            t0 = 0
            for grp_n in GROUPS:
                grp = list(range(t0, t0 + grp_n))
                ps = ppool.tile([128, grp_n * NB], F32, name="ps", tag="ps")
                for j, t in enumerate(grp):
                    ci, ct0 = chunk_of(t)
                    off = (t - ct0) * 128
                    nc.tensor.matmul(
                        ps[:, j * NB : (j + 1) * NB],
                        lhsT=qt_chunks[ci][:, off : off + 128],
                        rhs=embt_sb[:],
                        start=True,
                        stop=True,
                    )
                # s[l, b] = q[l, :] . emb[b, :]; copy on the scalar engine
                # keeps the DVE stream builds-only.
                nc.scalar.copy(
                    out=s_sb[:, t0 * NB : (t0 + grp_n) * NB],
                    in_=ps[:, : grp_n * NB],
                )

                for b0 in range(t0, t0 + grp_n, 2):
                    # One buffer covers s-tiles (b0, b0+1) = super-tile b0//2
                    # in the PACKED layout: the two tables sit at stride 4095
                    # (element 0 of each table is never read or written, so
                    # consecutive tables overlap by one column).  Partition p
                    # then holds rows p*2 and p*2+1 of the super-tile as ONE
                    # contiguous 32760B span -> one DMA descriptor per
                    # partition instead of two, amortizing the ~15ns/packet
                    # SDMA overhead (617 -> ~610 ns per 16380B equivalent).
                    U = tpool.tile([128, 2 * 4095 + 1], F32, name="U", tag="U")
                    for j, ti in enumerate([b0, b0 + 1]):
                        sb = ti * NB
                        base = j * 4095
                        hi = base + 2048
                        nc.vector.memset(U[:, hi : hi + 1], 0.0)
                        nc.vector.tensor_copy(
                            out=U[:, hi + 1 : hi + 2], in_=s_sb[:, sb : sb + 1]
                        )
                        for k in range(1, NB - 1):
                            nc.vector.tensor_scalar_add(
                                U[:, hi + 2**k : hi + 2 ** (k + 1)],
                                U[:, hi : hi + 2**k],
                                s_sb[:, sb + k : sb + k + 1],
                            )
                        nc.vector.tensor_scalar_add(
                            U[:, base + 1 : base + 2048],
                            U[:, hi + 1 : hi + 2048],
                            s_sb[:, sb + NB - 1 : sb + NB],
                        )
                    r0 = b0 * 128
                    src = U[:, 1 : 2 * 4095 + 1]
                    dst = out[r0 : r0 + 256, :].rearrange("(p r) m -> p (r m)", p=128)
                    nc.sync.dma_start(out=dst, in_=src)
                t0 += grp_n

    nc.finalize()
    return nc


def _install_trace_shim():
    """Make run_bass_kernel_spmd(trace=True) work under axon in this
    container: provide antenv.axon_hooks backed by ctypes calls into
    libaxon_pjrt.so, and skip the S3 artifact upload."""
    import contextlib
    import ctypes
    import types

    import antenv
    from concourse import bass_utils

    if getattr(antenv, "axon_hooks", None) is not None:
        return

    def _ntff_profile_via_ctypes(so_path):
        lib = ctypes.CDLL(so_path)
        if not hasattr(lib, "axon_start_nrt_profile"):
            return None
        lib.axon_start_nrt_profile.argtypes = [
            ctypes.POINTER(ctypes.c_int64),
            ctypes.c_size_t,
        ]
        lib.axon_start_nrt_profile.restype = ctypes.c_int64
        lib.axon_stop_nrt_profile.argtypes = [ctypes.c_char_p]
        lib.axon_stop_nrt_profile.restype = ctypes.c_int64

        @contextlib.contextmanager
        def _hook(output_dir, device_ids):
            import jax

            jax.devices()
            if device_ids:
                ids = (ctypes.c_int64 * len(device_ids))(*device_ids)
                rc = lib.axon_start_nrt_profile(ids, len(device_ids))
            else:
                rc = lib.axon_start_nrt_profile(None, 0)
            if rc != 0:
                raise RuntimeError(f"axon_start_nrt_profile rc={rc}")
            try:
                yield
            finally:
                n = lib.axon_stop_nrt_profile(str(output_dir).encode())
                print(f"trace shim: {n} ntff file(s) in {output_dir}", file=sys.stderr)

        return _hook

    mod = types.ModuleType("antenv.axon_hooks")
    state = {"hook": _ntff_profile_via_ctypes("/opt/axon/libaxon_pjrt.so")}
    mod.set_axon_ntff_profile_hook = lambda h: state.__setitem__("hook", h)
    mod.get_axon_ntff_profile_hook = lambda: state["hook"]
    sys.modules["antenv.axon_hooks"] = mod
    antenv.axon_hooks = mod
    bass_utils.upload_artifacts = lambda tmpdir: f"local://{tmpdir}"


def kernel(q, k, emb):
    global LAST_EXEC_TIME_NS
    trace = os.environ.get("KERNEL_TRACE", "") == "1"
    if trace:
        _install_trace_shim()

    nc = _build_nc()

    qr = np.asarray(q, dtype=np.float32).reshape(PAIRS, L, D)
    embT = np.ascontiguousarray(np.asarray(emb, dtype=np.float32).T)
    # Packed-layout column permutation: matmul for s-tile tt must place,
    # at PSUM partition p, the scalars of output row st*256 + p*2 + r
    # (st = tt//2, r = tt%2), so qT column tt*128+p holds that q row.
    perm = np.empty(ROWS, dtype=np.int64)
    p_ar = np.arange(128)
    for tt in range(NT):
        st, r = divmod(tt, 2)
        perm[tt * 128 + p_ar] = st * 256 + p_ar * 2 + r
    in_maps = []
    for c in range(NCORES):
        qc = qr[c * PPC : (c + 1) * PPC]  # [PPC, L, D]
        qTc = qc.transpose(2, 0, 1).reshape(D, ROWS)
        qTc = np.ascontiguousarray(qTc[:, perm])
        in_maps.append({"qT": qTc, "embT": embT})

    res = run_bass_kernel_spmd(nc, in_maps, core_ids=list(range(NCORES)), trace=trace)
    LAST_EXEC_TIME_NS = res.exec_time_ns

    out = np.empty((PAIRS, L, M), np.float32)
    for c in range(NCORES):
        out[c * PPC : (c + 1) * PPC] = res.results[c]["out"].reshape(PPC, L, M)
    return out.reshape(B, H, L, M)
